# revision 42
# baseline (speedup 1.0000x reference)
"""Trainium2 Bass kernel for nn_SubspaceLinopFactory (subspace NUDFT forward).

Math (reference):
  s[a,c,h,w] = x[a,h,w] * mps[c,h,w]
  y[a,c,k]   = sum_hw s * exp(-i*(ty_k*gy_h + tx_k*gx_w))   (separable NUDFT)
  z[t,c,k]   = sum_a phi[a,t] * y[a,c,k] * sqrt_dcf[k],  r = subsamp_idx[t]
Sharding: trajectory r -> core r (R == 8 == n_cores).

Device design (v3, per core):
  gy pairing: gy[h]=h-32; conjugate pairs (+g,-g), g=1..31, halve the
  k-elementwise work; gy=0 joins the plus block; gy=-32 is a small residual
  unit. Host stages paired image columns (spm), phase fractions (range-
  reduced phases in turns), dcf, and phi-combined +-selector weights.
  Pipeline per (unit u, k-chunk) with triple-buffered PSUM [128,2,512]:
    stage 1 (TensorE fp16, 64x64 quadrant-tiled, 4 concurrent matmuls):
        RE = [Pp(plus@cos) rows 0-63 | Qm(minus@sin) rows 64-127]
        IM = [Qp(plus@sin)           | Pm(minus@cos)]
    product: prod = (ytab*dcf) (*) bank  -- DVE direct from PSUM, or
        ScalarE fp16 cast + DVE/GpSimd 2x, per-unit mode
    reduce (TensorE, col-tiled 128x64 pair): z_re += selphi_re.T @ prod_re,
        z_im += selphi_im.T @ prod_im  (phi + h-reduction in one matmul,
        accumulated over all units in one PSUM super)
  Trig tables on device: ScalarE Sin(2*pi*frac) -> fp16.
  Output: one PSUM->SBUF fp16 copy + DMA of z [128, K]; host scatters rows
  (t-slot, c) into [T, C, K] complex64.
"""
import numpy as np

A, T, C, R, D, K, H, W = 3, 32, 4, 8, 2, 1024, 64, 64
N_CORES = 8
AC = A * C           # 12
MT = AC // 2         # 6 m-tiles
NU = MT + 1          # units incl. residual
NSLOT = 16           # t-slots per launch (M = 4*NSLOT = 64)
KC = 512

# per-unit product mode: 'V' direct DVE from PSUM, 'SV' ScalarE cast + DVE,
# 'SG' ScalarE cast + GpSimd product. Resid is unit 6.
UNIT_MODE = ['V', 'SV', 'SG', 'V', 'V', 'SG', 'SV']

_CACHE = {}


def _build_nc():
    import concourse.bacc as bacc
    import concourse.tile as tile
    import concourse.mybir as mybir

    AF = mybir.ActivationFunctionType
    OP = mybir.AluOpType
    F32 = mybir.dt.float32
    F16 = mybir.dt.float16
    TWO_PI = float(2 * np.pi)
    HALF_PI = float(np.pi / 2)

    nc = bacc.Bacc(None, target_bir_lowering=False)

    # inputs split by need-time: x fracs per chunk (cos-half staged
    # quarter-turn shifted so Sin args stay in [-pi, pi]), y-main fracs,
    # image pairs (spm), dcf, phi-selectors, resid fracs
    d_xf0 = nc.dram_tensor("xf0", [128, K], F16, kind="ExternalInput")
    d_md = nc.dram_tensor("md", [128, 2 * K], F16, kind="ExternalInput")
    d_ss = nc.dram_tensor("ss", [128, 896 + 896], F16, kind="ExternalInput")
    d_rx = nc.dram_tensor("rx", [128, 2 * K], F16, kind="ExternalInput")
    d_zout = nc.dram_tensor("zout", [128, K], F16, kind="ExternalOutput")

    with tile.TileContext(nc) as tc:
        with (
            tc.tile_pool(name="cst", bufs=1) as cst,
            tc.tile_pool(name="work", bufs=3) as work,
            tc.tile_pool(name="cwork", bufs=2) as cwork,
            tc.tile_pool(name="psS", bufs=3, space="PSUM") as psS,
            tc.tile_pool(name="psZ", bufs=1, space="PSUM") as psZ,
        ):
            # PE warm-up: dense junk matmuls on a never-written scratch tile
            # trip the HAM activity monitor to full clock while DMAs land.
            scratch = cst.tile([128, KC], F16)
            nc.vector.memzero(scratch[:])
            z = psZ.tile([128, K], F32)       # rows 0-63 z_re, 64-127 z_im
            for _ in range(9):
                nc.tensor.matmul(z[:, 0:KC], scratch[:, 0:128], scratch[:],
                                 start=True, stop=True, skip_group_check=True)
            xfrac = cst.tile([128, 2, KC], F16)     # chunk 0: [128, s|c, KC]
            spsel = cst.tile([128, 896 + 896], F16)
            mdt = cst.tile([128, 2, K], F16)    # [:,0,:]=ym, [:,1,:]=dcf
            rxt = cst.tile([128, 2, K], F16)    # [:,0,:]=yfr, [:,1,:]=xf1
            # priority-ordered DMAs, 2 per ring (sync=HWDGE, gpsimd=SWDGE)
            nc.sync.dma_start(xfrac[:], d_xf0[:].rearrange(
                "p (s k) -> p s k", s=2))
            nc.gpsimd.dma_start(spsel[:], d_ss[:])
            nc.sync.dma_start(mdt[:], d_md[:].rearrange(
                "p (c k) -> p c k", c=2))
            nc.gpsimd.dma_start(rxt[:], d_rx[:].rearrange(
                "p (c k) -> p c k", c=2))
            spm = spsel[:, 0:896]
            selphi = spsel[:, 896:896 + 896]
            ymf = mdt[:, 0, :]
            dcfb = mdt[:, 1, :]
            yfrac_r = rxt[:, 0, :]

            # trig tables: fp16 sin/cos via Sin(2*pi*frac [+ pi/2])
            xtab = cst.tile([128, 2, 2, KC], F16)  # [128, chunk, sin|cos, KC]
            ytab = cst.tile([128, 2, K], F16)   # [:,0,:]=main, [:,1,:]=resid
            # ytmd [128, 4, KC] = [Md-c0, Md-c0, Md-c1, Md-c1]
            ytmd = cst.tile([128, 4, KC], F16)
            ytrd = cst.tile([128, K], F16)
            zout_sb = cst.tile([128, K], F16)

            for kc in range(2):
                ks = slice(kc * KC, (kc + 1) * KC)
                xsrc = (xfrac[:] if kc == 0 else
                        rxt[:, 1, :].rearrange("p (s k) -> p s k", s=2))
                nc.scalar.activation(xtab[:, kc], xsrc, AF.Sin, scale=TWO_PI)
                nc.scalar.activation(ytab[:, 0, ks], ymf[:, ks],
                                     AF.Sin, scale=TWO_PI)
                for half in range(2):
                    nc.vector.tensor_tensor(ytmd[:, 2 * kc + half, :],
                                            ytab[:, 0, ks], dcfb[:, ks],
                                            OP.mult)
                nc.scalar.activation(ytab[:, 1, ks], yfrac_r[:, ks],
                                     AF.Sin, scale=TWO_PI)
                nc.vector.tensor_tensor(ytrd[:, ks], ytab[:, 1, ks],
                                        dcfb[:, ks], OP.mult)

                for u in range(NU):
                    mode = UNIT_MODE[u]
                    # PSUM per unit-chunk: [128, RE|IM, KC]
                    bank = psS.tile([128, 2, KC], F32, tag="bank")
                    # rhs A: rows 0-63 cos, 64-127 sin; rhs B: sin | cos
                    # (the xfrac bottom half is staged swapped). lhsT is the
                    # block-diagonal [s+ 0; 0 s-] so one full 128-contraction
                    # matmul yields [Pp|Qm] (RE) resp. [Qp|Pm] (IM).
                    xtabA = xtab[:, kc, 1, :]
                    xtabB = xtab[:, kc, 0, :]
                    if u < MT:
                        cb = u * 128
                        nc.tensor.matmul(bank[:, 0, :],
                                         spm[:, cb:cb + 128], xtabA,
                                         start=True, stop=True)   # Pp | Qm
                        nc.tensor.matmul(bank[:, 1, :],
                                         spm[:, cb:cb + 128], xtabB,
                                         start=True, stop=True)   # Qp | Pm
                        ncols, tsl = 2, ytmd[:, 2 * kc:2 * kc + 2, :]
                        bsl = bank[:, 0:2, :]
                    else:
                        nc.tensor.matmul(bank[0:88, 0, :],
                                         spm[:, 768:856], xtabA,
                                         start=True, stop=True)   # P0 | Q0
                        ncols, tsl = 1, ytrd[:, ks]
                        bsl = bank[:, 0, :]

                    prod = work.tile([128, 2, KC], F16, tag="prod")
                    psl = prod[:, 0:2, :] if ncols == 2 else prod[:, 0, :]
                    if mode == 'V':
                        nc.vector.tensor_tensor(psl, bsl, tsl, OP.mult)
                    else:
                        cast = cwork.tile([128, 2, KC], F16, tag="cast")
                        csl = cast[:, 0:2, :] if ncols == 2 else cast[:, 0, :]
                        nc.scalar.copy(csl, bsl)
                        if mode == 'SV':
                            nc.vector.tensor_tensor(psl, csl, tsl, OP.mult)
                        else:
                            nc.gpsimd.tensor_tensor(psl, csl, tsl, OP.mult)

                    # fused h-reduce + phi matmuls (col-tiled pair)
                    st = (u == 0)
                    sp = (u == NU - 1)
                    re_rhs = prod[:, 0, :]
                    im_rhs = prod[:, 1, :] if u < MT else prod[:, 0, :]
                    nc.tensor.matmul(z[0:64, ks],
                                     selphi[:, (2 * u) * 64:(2 * u + 1) * 64],
                                     re_rhs, start=st, stop=sp,
                                     skip_group_check=True)
                    nc.tensor.matmul(z[64:128, ks],
                                     selphi[:, (2 * u + 1) * 64:(2 * u + 2) * 64],
                                     im_rhs, start=st, stop=sp,
                                     skip_group_check=True)

                # per-chunk output: copy + DMA overlap the next chunk
                nc.scalar.copy(zout_sb[:, ks], z[:, ks])
                nc.gpsimd.dma_start(d_zout[:, ks], zout_sb[:, ks])

    nc.finalize()
    return nc


def _get_nc():
    if "nc" not in _CACHE:
        _CACHE["nc"] = _build_nc()
    return _CACHE["nc"]


def _stage_core(r, x, trj, phi, mps, sqrt_dcf, owned_ts):
    """Host staging for core r: layout/pairing of inputs, phase fractions
    (range-reduced phases in turns), and phi-signed selector weights."""
    f16 = np.float16
    ty = trj[r, 0, :].astype(np.float64)
    tx = trj[r, 1, :].astype(np.float64)
    inv2pi = 1.0 / (2 * np.pi)

    def frac(v):
        return v - np.round(v)

    gx = (np.arange(W) - W // 2).astype(np.float64)
    mx = np.outer(gx, tx) * inv2pi
    # chunk-major: [128, chunk, A|B, KC]; bottom partition half swapped so
    # that slice A = [cos | sin] and slice B = [sin | cos] across halves
    xf = np.empty((128, 2, 2, KC), np.float64)
    for kc in range(2):
        ks = slice(kc * KC, (kc + 1) * KC)
        xf[:64, kc, 0, :] = frac(mx[:, ks])
        xf[:64, kc, 1, :] = frac(mx[:, ks] + 0.25)
        xf[64:, kc, 0, :] = xf[:64, kc, 1, :]
        xf[64:, kc, 1, :] = xf[:64, kc, 0, :]

    g = np.arange(32).astype(np.float64)
    my = np.outer(g, ty) * inv2pi
    yf = np.zeros((128, 2, K), np.float64)
    yf[0:32, 0, :] = frac(my + 0.25)
    yf[32:64, 0, :] = yf[0:32, 0, :]
    yf[64:96, 0, :] = frac(my)
    yf[96:128, 0, :] = yf[64:96, 0, :]
    m32 = 32.0 * ty * inv2pi
    yf[0:12, 1, :] = frac(m32 + 0.25)
    yf[12:24, 1, :] = frac(m32)
    yf[64:76, 1, :] = frac(m32)
    yf[76:88, 1, :] = frac(m32 + 0.25)

    dcfb = np.broadcast_to(sqrt_dcf[r].astype(f16), (128, K))

    # block-diagonal lhsT columns: plus-pairs on the top w-half (contracting
    # the cos rows of rhs A), minus-pairs on the bottom (sin rows)
    s = (x[:, None, :, :] * mps[None, :, :, :]).reshape(AC, H, W)
    sp = s[:, 33:64, :]                     # gy = +1..+31
    sm = s[:, 31:0:-1, :]                   # gy = -1..-31
    spl = np.zeros((128, 896), np.float64)
    for j in range(MT):
        for i, ac in enumerate((2 * j, 2 * j + 1)):
            pc = j * 128 + i * 32
            mc = j * 128 + 64 + i * 32
            spl[:64, pc] = s[ac, 32, :]
            spl[:64, pc + 1:pc + 32] = (sp[ac] + sm[ac]).T
            spl[64:, mc + 1:mc + 32] = (sp[ac] - sm[ac]).T
    spl[:64, 768:780] = s[:, 0, :].T        # P0 rows (out 0-23)
    spl[:64, 780:792] = s[:, 0, :].T
    spl[64:, 832:844] = s[:, 0, :].T        # Q0 rows (out 64-87)
    spl[64:, 844:856] = s[:, 0, :].T

    selphi = np.zeros((128, 14 * 64), np.float64)
    for j in range(MT):
        sre = selphi[:, (2 * j) * 64:(2 * j) * 64 + 64]
        sim = selphi[:, (2 * j + 1) * 64:(2 * j + 1) * 64 + 64]
        for i, ac in enumerate((2 * j, 2 * j + 1)):
            a, c = divmod(ac, C)
            for slot, t in enumerate(owned_ts):
                m = 4 * slot + c
                p = phi[a, t]
                sre[i * 32:(i + 1) * 32, m] = p
                sre[64 + i * 32:96 + i * 32, m] = -p
                sim[i * 32:(i + 1) * 32, m] = -p
                sim[64 + i * 32:96 + i * 32, m] = -p
    sre = selphi[:, 12 * 64:13 * 64]
    sim = selphi[:, 13 * 64:14 * 64]
    for ac in range(AC):
        a, c = divmod(ac, C)
        for slot, t in enumerate(owned_ts):
            m = 4 * slot + c
            p = phi[a, t]
            sre[ac, m] = p
            sre[64 + ac, m] = p
            sim[12 + ac, m] = p
            sim[76 + ac, m] = -p

    return {
        "xf0": xf[:, 0].reshape(128, K).astype(f16),
        "md": np.concatenate([yf[:, 0, :],
                              dcfb.astype(np.float64)], axis=1).astype(f16),
        "ss": np.concatenate([spl, selphi], axis=1).astype(f16),
        "rx": np.concatenate([yf[:, 1, :],
                              xf[:, 1].reshape(128, K)], axis=1).astype(f16),
    }


def kernel(x, trj, phi, mps, sqrt_dcf, subsamp_idx, _trace=False):
    from concourse.bass_utils import run_bass_kernel_spmd

    x = np.asarray(x, dtype=np.float32)
    trj = np.asarray(trj, dtype=np.float32)
    phi = np.asarray(phi, dtype=np.float32)
    mps = np.asarray(mps, dtype=np.float32)
    sqrt_dcf = np.asarray(sqrt_dcf, dtype=np.float32)
    idx = np.asarray(subsamp_idx).astype(np.int64)

    nc = _get_nc()
    owned = {r: [t for t in range(T) if idx[t] == r] for r in range(R)}
    out = np.empty((T, C, K), dtype=np.complex64)
    launches = max(1, max((len(v) + NSLOT - 1) // NSLOT
                          for v in owned.values()))
    for li in range(launches):
        batch = {r: owned[r][li * NSLOT:(li + 1) * NSLOT] for r in range(R)}
        in_maps = [
            _stage_core(r, x, trj, phi, mps, sqrt_dcf, batch[r])
            for r in range(N_CORES)
        ]
        res = run_bass_kernel_spmd(nc, in_maps, core_ids=list(range(N_CORES)),
                                   trace=_trace)
        for r in range(N_CORES):
            if not batch[r]:
                continue
            zout = res.results[r]["zout"].astype(np.float32)
            for slot, t in enumerate(batch[r]):
                for c in range(C):
                    out[t, c, :] = (zout[4 * slot + c]
                                    + 1j * zout[64 + 4 * slot + c])
        if _trace:
            kernel._last_results = res
    return out
